# revision 2
# baseline (speedup 1.0000x reference)
"""Trainium2 Bass kernel for nn_Linear_80874234183916.

Computes y = x @ w_eff.T + bias where w_eff keeps only the weight entries
whose |w| is >= the k-th largest magnitude, k = max_iter = n/2 (the budgeted
approximate matmul of the reference: threshold = median of |w|).

Sharding: tensor-parallel over out_features across 8 NeuronCores — each core
owns a 512-column slice of the output and computes x @ w_slice_eff.T +
bias_slice; x is replicated and streamed. The 8 per-core [8192, 512] slices
are concatenated on the out dim.

Numerics: the mask (|w| >= thresh) is computed on host in full fp32 — the
comparison must NOT be done on rounded weights (boundary flips near the
median would add ~4% rel err). The masked weights and x are then rounded to
bf16 for the PE (fp32 PSUM accumulation), giving ~1.5e-3 rel err, well
inside the 2e-2 gate. bf16 operands let walrus emit separate
LDWEIGHTS/MATMUL (pipelined through the PE background weight buffer + FWL)
instead of the serialized self-loading fp32r matmul, and halve HBM traffic.

Host-side work: threshold (np.partition), masking+cast, layout prep
(transpose/tiling so every device DMA is contiguous full-partition), final
concat. All O(N*K*M) matmul work and the bias add run on device.

Per-core structure (x-stationary): 64 token tiles; per tile a 32-matmul
accumulation chain (stationary = x tile [ki,t] bf16, moving = w slice
[ki, 512] bf16) into a PSUM bank, `ilv` chains interleaved across banks;
PSUM + bias -> SBUF on DVE, DMA out.
"""

import numpy as np
import ml_dtypes

import concourse.bass as bass
import concourse.mybir as mybir
import concourse.tile as tile
from concourse import bacc
from concourse.bass_utils import run_bass_kernel_spmd

N_TOK = 8192
IN_F = 4096
OUT_F = 4096
N_CORES = 8
O_S = OUT_F // N_CORES  # 512 out-features per core
P = 128
KO = IN_F // P          # 32 k-chunks
TT = N_TOK // P         # 64 token tiles
X_BUFS = 6
MAX_ITER = IN_F * OUT_F // 2

dt = mybir.dt
BF16 = ml_dtypes.bfloat16


def _build(thresh: float = 0.0, reps: int = 1, ilv: int = 2):
    """Build the per-core Bass program (SPMD: same NEFF, per-core data).

    reps>1 repeats the token-tile loop (timing experiments only); ilv
    interleaves that many accumulation chains across PSUM banks. thresh is
    unused (masking is host-side) but kept for test.py compatibility.
    """
    nc = bacc.Bacc("TRN2", target_bir_lowering=False, debug=False)

    # Host pre-tiled layouts (see _prep_inputs for the packing):
    #   xt[tt, ki, ko, t] = x[tt*128 + t, ko*128 + ki]          (bf16)
    #   wt[ki, ko, n]     = w_eff_slice[n, ko*128 + ki]         (bf16, masked)
    xt = nc.dram_tensor("xt", [TT, P, KO, P], dt.bfloat16, kind="ExternalInput").ap()
    wt = nc.dram_tensor("wt", [P, KO, O_S], dt.bfloat16, kind="ExternalInput").ap()
    bb = nc.dram_tensor("bb", [P, O_S], dt.float32, kind="ExternalInput").ap()
    y = nc.dram_tensor("y", [N_TOK, O_S], dt.float32, kind="ExternalOutput").ap()

    with tile.TileContext(nc) as tc:
        with (
            tc.tile_pool(name="wpool", bufs=1) as wpool,
            tc.tile_pool(name="xpool", bufs=X_BUFS) as xpool,
            tc.tile_pool(name="opool", bufs=3) as opool,
            tc.tile_pool(name="cpool", bufs=1) as cpool,
            tc.tile_pool(name="pspool", bufs=8, space="PSUM") as ps,
        ):
            bias_sb = cpool.tile([P, O_S], dt.float32, tag="bias")
            nc.sync.dma_start(bias_sb[:], bb)

            wm_sb = wpool.tile([P, KO, O_S], dt.bfloat16, tag="wm")
            nc.sync.dma_start(wm_sb[:], wt)

            for _rep in range(reps):
                for gi in range(0, TT, ilv):
                    grp = range(gi, min(gi + ilv, TT))
                    xg = []
                    for tt in grp:
                        x_sb = xpool.tile([P, KO, P], dt.bfloat16, tag="x",
                                          name=f"x{tt}")
                        nc.sync.dma_start(x_sb[:], xt[tt])
                        xg.append(x_sb)
                    psg = [
                        ps.tile([P, O_S], dt.float32, tag="ps", name=f"psg{t}")
                        for t in range(len(xg))
                    ]
                    for ko in range(KO):
                        for gj in range(len(xg)):
                            nc.tensor.matmul(
                                psg[gj][:],
                                xg[gj][:, ko],
                                wm_sb[:, ko],
                                start=(ko == 0),
                                stop=(ko == KO - 1),
                            )
                    for gj, tt in enumerate(grp):
                        out_sb = opool.tile([P, O_S], dt.float32, tag="out",
                                            name=f"out{tt}")
                        nc.vector.tensor_add(out_sb[:], psg[gj][:], bias_sb[:])
                        nc.sync.dma_start(y[tt * P : (tt + 1) * P, :], out_sb[:])

    nc.compile()
    return nc


def _prep_inputs(x, weight, bias):
    """Host-side: threshold (fp32), mask+cast to bf16, DMA-friendly tiling."""
    flat_abs = np.abs(weight.reshape(-1))
    k = flat_abs.size - MAX_ITER
    thresh = float(np.partition(flat_abs, k)[k])

    # Mask in fp32 (exact vs reference), THEN round to bf16.
    w_eff = (weight * (np.abs(weight) >= thresh)).astype(BF16)

    # xt[tt, ki, ko, t] = x[tt*128+t, ko*128+ki]
    xt = np.ascontiguousarray(
        x.reshape(TT, P, KO, P).transpose(0, 3, 2, 1).astype(BF16)
    )

    in_maps = []
    for c in range(N_CORES):
        w_s = w_eff[c * O_S : (c + 1) * O_S]  # [O_S, IN_F] bf16
        # wt[ki, ko, n] = w_s[n, ko*128+ki]
        wt = np.ascontiguousarray(w_s.reshape(O_S, KO, P).transpose(2, 1, 0))
        bb = np.ascontiguousarray(
            np.broadcast_to(bias[c * O_S : (c + 1) * O_S], (P, O_S))
        ).astype(np.float32)
        in_maps.append({"xt": xt, "wt": wt, "bb": bb})
    return thresh, in_maps


def _run(x, weight, bias, **run_kwargs):
    x = np.asarray(x, dtype=np.float32)
    weight = np.asarray(weight, dtype=np.float32)
    bias = np.asarray(bias, dtype=np.float32)
    assert x.shape == (N_TOK, IN_F) and weight.shape == (OUT_F, IN_F)

    thresh, in_maps = _prep_inputs(x, weight, bias)
    nc = _build(thresh)
    res = run_bass_kernel_spmd(
        nc, in_maps, core_ids=list(range(N_CORES)), **run_kwargs
    )
    y = np.concatenate([r["y"] for r in res.results], axis=1)
    return y, res


def kernel(x, weight, bias):
    y, _ = _run(x, weight, bias)
    return y


# revision 3
# speedup vs baseline: 1.0511x; 1.0511x over previous
"""v2: w-stationary stream kernel for nn_Linear_80874234183916.

Same problem as kernel.py, but the PE stationary operand is a 128x128 block
of the (masked, bf16) weight slice and the moving operand is a 512-token
slab of x^T. Each LDWEIGHTS is reused by 2 consecutive matmuls, and bf16
FWL + the PE background weight buffer hide the rest. Output lands in PSUM
transposed ([out_feature, token]); bias is added on the scalar engine as a
per-partition scalar during the PSUM->SBUF drain, freeing the DVE entirely.

Per-core structure (TP over out_features, O_S=512 = 4 nb-blocks of 128):
  - w slice resident in SBUF as 8 chunked bf16 tiles; x^T slabs of 512
    tokens arrive as 4 sub-tiles of [128ki, 8ko, 512t] (1 MiB) each, all
    DMAs issued in PE consumption order (w chunk, then the x chunks that
    need it) so the first matmul starts ~1.5 MiB into the stream.
  - per group of 2 chunks, two passes (nb in {0,1} then {2,3}); each pass
    holds 4 PSUM banks with 32-matmul accumulation chains while the other
    4 banks drain through ACT (bias add) -> SBUF -> DMA to y^T.
"""

import numpy as np
import ml_dtypes

import concourse.bass as bass
import concourse.mybir as mybir
import concourse.tile as tile
from concourse import bacc
from concourse.bass_utils import run_bass_kernel_spmd

N_TOK = 8192
IN_F = 4096
OUT_F = 4096
N_CORES = 8
O_S = OUT_F // N_CORES  # 512 out-features per core
P = 128
KO = IN_F // P          # 32 k-chunks
NB = O_S // P           # 4 out-feature blocks
TCH = 512               # tokens per chunk (moving dim)
CT = N_TOK // TCH       # 16 chunks
TT = 64                 # kept for test.py tau scaling compat
WCH = 4                 # ko per w tile chunk (8 chunks of 512 KiB)
XCH = 8                 # ko per x sub-tile (4 sub-tiles of 1 MiB per slab)
MAX_ITER = IN_F * OUT_F // 2

dt = mybir.dt
BF16 = ml_dtypes.bfloat16


def _build(thresh: float = 0.0, reps: int = 1, x_bufs: int = 16):
    nc = bacc.Bacc("TRN2", target_bir_lowering=False, debug=False)

    # Host layouts (see _prep_inputs):
    #   xq[ct, ki, ko, t] = x[ct*512 + t, ko*128 + ki]            (bf16)
    #   wq[ki, ko, n]     = w_eff_slice[n, ko*128 + ki]           (bf16)
    #   bt[p, nb]         = bias_slice[nb*128 + p]                (f32)
    #   yt[nb, p, tok]    = y[tok, c*512 + nb*128 + p]            (f32 out)
    xq = nc.dram_tensor("xq", [CT, P, KO, TCH], dt.bfloat16, kind="ExternalInput").ap()
    wq = nc.dram_tensor("wq", [P, KO, O_S], dt.bfloat16, kind="ExternalInput").ap()
    bt = nc.dram_tensor("bt", [P, NB], dt.float32, kind="ExternalInput").ap()
    yt = nc.dram_tensor("yt", [NB, P, N_TOK], dt.float32, kind="ExternalOutput").ap()

    n_wch = KO // WCH
    n_xch = KO // XCH

    with tile.TileContext(nc) as tc:
        with (
            tc.tile_pool(name="wpool", bufs=n_wch) as wpool,
            tc.tile_pool(name="xpool", bufs=x_bufs) as xpool,
            tc.tile_pool(name="opool", bufs=8) as opool,
            tc.tile_pool(name="cpool", bufs=1) as cpool,
            tc.tile_pool(name="pspool", bufs=8, space="PSUM") as ps,
        ):
            wms = [None] * n_wch

            def load_w(wc):
                wm = wpool.tile([P, WCH, O_S], dt.bfloat16, tag="wm",
                                name=f"wm{wc}")
                nc.sync.dma_start(wm[:], wq[:, wc * WCH : (wc + 1) * WCH])
                wms[wc] = wm

            def wslice(ko, nb):
                return wms[ko // WCH][:, ko % WCH, nb * P : (nb + 1) * P]

            xs = {}

            def load_x(rep, ct, xc):
                x_sb = xpool.tile([P, XCH, TCH], dt.bfloat16, tag="x",
                                  name=f"x{rep}_{ct}_{xc}")
                nc.sync.dma_start(
                    x_sb[:], xq[ct, :, xc * XCH : (xc + 1) * XCH]
                )
                xs[ct, xc] = x_sb

            def xslice(ct, ko):
                return xs[ct, ko // XCH][:, ko % XCH]

            # Prologue: interleave w chunks with the first group's x chunks
            # in PE consumption order (ko-major).
            for xc in range(n_xch):
                load_w(2 * xc)
                load_w(2 * xc + 1)
                load_x(0, 0, xc)
                load_x(0, 1, xc)
            bias_sb = cpool.tile([P, NB], dt.float32, tag="bias")
            nc.sync.dma_start(bias_sb[:], bt)

            for _rep in range(reps):
                for g in range(CT // 2):
                    cts = (2 * g, 2 * g + 1)
                    if not (_rep == 0 and g == 0):
                        for xc in range(n_xch):
                            for ct in cts:
                                load_x(_rep, ct, xc)
                    for half in range(2):
                        nbs = (2 * half, 2 * half + 1)
                        pss = {}
                        for nb in nbs:
                            for j in range(2):
                                pss[nb, j] = ps.tile(
                                    [P, TCH], dt.float32, tag="ps",
                                    name=f"ps{g}_{nb}_{j}",
                                )
                        for ko in range(KO):
                            for nb in nbs:
                                for j, ct in enumerate(cts):
                                    nc.tensor.matmul(
                                        pss[nb, j][:],
                                        wslice(ko, nb),
                                        xslice(ct, ko),
                                        start=(ko == 0),
                                        stop=(ko == KO - 1),
                                    )
                        for nb in nbs:
                            for j, ct in enumerate(cts):
                                o = opool.tile([P, TCH], dt.float32, tag="out",
                                               name=f"o{g}_{nb}_{j}")
                                nc.scalar.add(o[:], pss[nb, j][:],
                                              bias_sb[:, nb : nb + 1])
                                nc.sync.dma_start(
                                    yt[nb, :, ct * TCH : (ct + 1) * TCH], o[:]
                                )
                    for ct in cts:
                        for xc in range(n_xch):
                            del xs[ct, xc]

    nc.compile()
    return nc


def _prep_inputs(x, weight, bias):
    """Host-side: threshold (fp32), mask+cast to bf16, DMA-friendly tiling."""
    flat_abs = np.abs(weight.reshape(-1))
    k = flat_abs.size - MAX_ITER
    thresh = float(np.partition(flat_abs, k)[k])

    w_eff = (weight * (np.abs(weight) >= thresh)).astype(BF16)

    # xq[ct, ki, ko, t] = x[ct*512+t, ko*128+ki]
    xq = np.ascontiguousarray(
        x.reshape(CT, TCH, KO, P).transpose(0, 3, 2, 1).astype(BF16)
    )

    in_maps = []
    for c in range(N_CORES):
        w_s = w_eff[c * O_S : (c + 1) * O_S]  # [O_S, IN_F] bf16
        wq = np.ascontiguousarray(w_s.reshape(O_S, KO, P).transpose(2, 1, 0))
        bt = np.ascontiguousarray(
            bias[c * O_S : (c + 1) * O_S].reshape(NB, P).T
        ).astype(np.float32)
        in_maps.append({"xq": xq, "wq": wq, "bt": bt})
    return thresh, in_maps


def _run(x, weight, bias, **run_kwargs):
    x = np.asarray(x, dtype=np.float32)
    weight = np.asarray(weight, dtype=np.float32)
    bias = np.asarray(bias, dtype=np.float32)
    assert x.shape == (N_TOK, IN_F) and weight.shape == (OUT_F, IN_F)

    thresh, in_maps = _prep_inputs(x, weight, bias)
    nc = _build(thresh)
    res = run_bass_kernel_spmd(
        nc, in_maps, core_ids=list(range(N_CORES)), **run_kwargs
    )
    # yt[nb, p, tok] per core -> y[tok, c*512 + nb*128 + p]
    y = np.concatenate(
        [r["yt"].reshape(O_S, N_TOK).T for r in res.results], axis=1
    )
    return np.ascontiguousarray(y), res


def kernel(x, weight, bias):
    y, _ = _run(x, weight, bias)
    return y
